# revision 1
# baseline (speedup 1.0000x reference)
"""MaxUnpooling2D scatter-add kernel for Trainium2 (8 NeuronCores).

Problem: updates/mask [32,112,112,64] f32/int32 -> out [32,224,224,64] f32,
out[b, y, x, c] += updates[b, h, w, c] with y/x decoded from mask (random
full-range indices, duplicates summed).

Strategy (all scatter work on device):
  - Shard by batch-pair: 16 pairs x 2 column-halves = 32 work units over
    8 cores x 4 sequential invocations of ONE compiled module.
  - Data laid out plane-major ([batch-local, channel] -> partition) so each
    indirect-DMA call scatters 128 elements from 128 DISJOINT output planes:
    offsets within a call are structurally unique (no duplicate races).
  - Scatter via gpsimd indirect DMA with CCE accumulate (compute_op=add);
    calls on the same SWDGE queue serialize, so cross-call duplicate indices
    accumulate exactly (verified on hardware).
  - ExternalOutput buffers are pre-zeroed by the runtime; partial outputs of
    the two column-halves of a pair are summed on host.
"""
import numpy as np

import concourse.bacc as bacc
import concourse.bass as bass
import concourse.mybir as mybir
import concourse.tile as tile
from concourse.bass_utils import run_bass_kernel_spmd

B, H, W, C = 32, 112, 112, 64
OUT_HW = (2 * H) * (2 * W)            # 224*224
PLANE = OUT_HW                         # bins per (b, c) plane = 50176
BATCH_BINS = OUT_HW * C                # 3211264
PAIR_BINS = 2 * BATCH_BINS             # 6422528
NPOS = H * W                           # 12544 source positions per batch
HALF = NPOS // 2                       # 6272 columns per module invocation
N_CORES = 8

_cached_nc = None
_cached_runner = None


def _make_runner(nc):
    """Jit-once multi-core runner with device-resident zero output buffers.

    run_bass_via_pjrt ships fresh host zero-buffers (24.5 MB/core) for output
    donation on EVERY invocation and re-traces the jit; over 4 invocations
    that is ~784 MB of host->device transfer through the axon tunnel. Here
    the zero operands live on device once and the jitted executable is
    reused (no donation, so the zero buffers are not consumed).
    """
    import jax
    import jax.numpy as jnp
    from jax.experimental.shard_map import shard_map
    from jax.sharding import Mesh, PartitionSpec
    import concourse.mybir as _mb
    from concourse.bass2jax import (
        _bass_exec_p,
        install_neuronx_cc_hook,
        partition_id_tensor,
    )

    install_neuronx_cc_hook()
    partition_name = nc.partition_id_tensor.name if nc.partition_id_tensor else None
    in_names, out_names, out_avals, zero_outs = [], [], [], []
    for alloc in nc.m.functions[0].allocations:
        if not isinstance(alloc, _mb.MemoryLocationSet):
            continue
        name = alloc.memorylocations[0].name
        if alloc.kind == "ExternalInput":
            if name != partition_name:
                in_names.append(name)
        elif alloc.kind == "ExternalOutput":
            shape = tuple(alloc.tensor_shape)
            dtype = _mb.dt.np(alloc.dtype)
            out_names.append(name)
            out_avals.append(jax.core.ShapedArray(shape, dtype))
            zero_outs.append(np.zeros(shape, dtype))
    n_params = len(in_names)
    all_names = in_names + out_names
    if partition_name is not None:
        all_names.append(partition_name)

    def _body(*args):
        operands = list(args)
        if partition_name is not None:
            operands.append(partition_id_tensor())
        outs = _bass_exec_p.bind(
            *operands,
            out_avals=tuple(out_avals),
            in_names=tuple(all_names),
            out_names=tuple(out_names),
            lowering_input_output_aliases=(),
            sim_require_finite=True,
            sim_require_nnan=True,
            nc=nc,
        )
        return tuple(outs)

    devices = jax.devices()[:N_CORES]
    mesh = Mesh(np.asarray(devices), ("core",))
    nin = n_params + len(out_names)
    donate = tuple(range(n_params, nin))
    sharded = jax.jit(
        shard_map(
            _body,
            mesh=mesh,
            in_specs=(PartitionSpec("core"),) * nin,
            out_specs=(PartitionSpec("core"),) * len(out_names),
            check_rep=False,
        ),
        donate_argnums=donate,
        keep_unused=True,
    )
    sharding = jax.sharding.NamedSharding(mesh, PartitionSpec("core"))
    # zero output buffers are donated (consumed) per call; manufacture them
    # ON DEVICE each call instead of shipping 24.5 MB/core of host zeros
    zshapes = [
        ((N_CORES * z.shape[0], *z.shape[1:]), z.dtype.type) for z in zero_outs
    ]
    zeros_factory = jax.jit(
        lambda: tuple(jnp.zeros(s, d) for s, d in zshapes),
        out_shardings=tuple(sharding for _ in zshapes),
    )

    def run(in_maps, init=None, fetch=True):
        """init: device arrays to accumulate into (donated); None -> zeros.
        fetch=False returns the raw device arrays (for chaining)."""
        concat_in = [
            np.concatenate([np.asarray(in_maps[c][nm]) for c in range(N_CORES)], axis=0)
            for nm in in_names
        ]
        concat_in = [jax.device_put(a, sharding) for a in concat_in]
        inits = zeros_factory() if init is None else init
        out_arrs = sharded(*concat_in, *inits)
        if not fetch:
            return out_arrs
        return [
            {
                nm: np.asarray(out_arrs[i]).reshape(N_CORES, *out_avals[i].shape)[c]
                for i, nm in enumerate(out_names)
            }
            for c in range(N_CORES)
        ]

    return run


def _build_module():
    """One core's work unit: scatter [128 planes x HALF cols] into a
    2-batch output with exact duplicate accumulation."""
    nc = bacc.Bacc("TRN2", target_bir_lowering=False, debug=False)
    val_d = nc.dram_tensor("val", [128, HALF], mybir.dt.float32, kind="ExternalInput")
    msk_d = nc.dram_tensor("msk", [128, HALF], mybir.dt.int32, kind="ExternalInput")
    basec_d = nc.dram_tensor("basec", [128, 1], mybir.dt.int32, kind="ExternalInput")
    bbase_d = nc.dram_tensor("bbase", [128, 1], mybir.dt.int32, kind="ExternalInput")
    neg64_d = nc.dram_tensor("neg64", [128, 1], mybir.dt.int32, kind="ExternalInput")
    out_d = nc.dram_tensor("out", [PAIR_BINS, 1], mybir.dt.float32, kind="ExternalOutput")

    with tile.TileContext(nc) as tc:
        with tc.tile_pool(name="sbuf", bufs=1) as pool:
            val = pool.tile([128, HALF], mybir.dt.float32)
            msk = pool.tile([128, HALF], mybir.dt.int32)
            off = pool.tile([128, HALF], mybir.dt.int32)
            basec = pool.tile([128, 1], mybir.dt.int32)
            bbase = pool.tile([128, 1], mybir.dt.int32)
            neg64 = pool.tile([128, 1], mybir.dt.int32)
            nc.sync.dma_start(out=val[:], in_=val_d[:])
            nc.sync.dma_start(out=msk[:], in_=msk_d[:])
            nc.sync.dma_start(out=basec[:], in_=basec_d[:])
            nc.sync.dma_start(out=bbase[:], in_=bbase_d[:])
            nc.sync.dma_start(out=neg64[:], in_=neg64_d[:])
            # off = ((msk & -64) | c) + batch_local*BATCH_BINS
            # low 6 bits of (msk & -64) are zero, c < 64 -> OR == ADD there
            nc.vector.scalar_tensor_tensor(
                out=off[:],
                in0=msk[:],
                scalar=neg64[:, 0:1],
                in1=basec[:, 0:1].to_broadcast([128, HALF]),
                op0=mybir.AluOpType.bitwise_and,
                op1=mybir.AluOpType.bitwise_or,
            )
            nc.vector.tensor_tensor(
                out=off[:],
                in0=off[:],
                in1=bbase[:, 0:1].to_broadcast([128, HALF]),
                op=mybir.AluOpType.add,
            )
            for j in range(HALF):
                nc.gpsimd.indirect_dma_start(
                    out=out_d[:],
                    out_offset=bass.IndirectOffsetOnAxis(ap=off[:, j:j + 1], axis=0),
                    in_=val[:, j:j + 1],
                    in_offset=None,
                    compute_op=mybir.AluOpType.add,
                )
    nc.compile()
    return nc


def _get_module():
    global _cached_nc
    if _cached_nc is None:
        _cached_nc = _build_module()
    return _cached_nc


def kernel(updates: np.ndarray, mask: np.ndarray) -> np.ndarray:
    assert updates.shape == (B, H, W, C) and mask.shape == (B, H, W, C)
    updates = np.ascontiguousarray(updates, dtype=np.float32)
    mask = np.ascontiguousarray(mask, dtype=np.int32)

    # plane-major: [B, C, NPOS]
    upd_t = np.ascontiguousarray(updates.reshape(B, NPOS, C).transpose(0, 2, 1))
    msk_t = np.ascontiguousarray(mask.reshape(B, NPOS, C).transpose(0, 2, 1))

    # per-partition constants: partition p = (batch_local = p//64, c = p%64)
    p = np.arange(128, dtype=np.int32)
    basec = (p % C).reshape(128, 1).astype(np.int32)
    bbase = ((p // C) * BATCH_BINS).reshape(128, 1).astype(np.int32)
    neg64 = np.full((128, 1), -64, dtype=np.int32)

    nc = _get_module()

    global _cached_runner
    if _cached_runner is None:
        _cached_runner = _make_runner(nc)

    # 32 work units: (pair bp in 0..16) x (half h in 0..2); core = bp % 8.
    # The two column-halves of a pair are CHAINED on device: half 0 scatters
    # into zeros, half 1 donates half 0's output as its initial buffer and
    # accumulates on top (CCE add), so each pair's output crosses the axon
    # link only once.
    def in_maps_for(grp, h):
        in_maps = []
        for core in range(N_CORES):
            b0 = 2 * (grp * 8 + core)
            val = upd_t[b0:b0 + 2, :, h * HALF:(h + 1) * HALF].reshape(128, HALF)
            msk = msk_t[b0:b0 + 2, :, h * HALF:(h + 1) * HALF].reshape(128, HALF)
            in_maps.append({
                "val": np.ascontiguousarray(val),
                "msk": np.ascontiguousarray(msk),
                "basec": basec,
                "bbase": bbase,
                "neg64": neg64,
            })
        return in_maps

    # launch both group chains asynchronously (jax dispatch is non-blocking),
    # fetch at the end so H2D, device work, and D2H pipeline
    pending = []
    for grp in range(2):
        part = _cached_runner(in_maps_for(grp, 0), fetch=False)
        pending.append(_cached_runner(in_maps_for(grp, 1), init=part, fetch=False))

    out = np.empty((B, 2 * H, 2 * W, C), dtype=np.float32)
    for grp in range(2):
        arr = np.asarray(pending[grp][0]).reshape(N_CORES, PAIR_BINS)
        for core in range(N_CORES):
            bp = grp * 8 + core
            out[2 * bp:2 * bp + 2] = arr[core].reshape(2, 2 * H, 2 * W, C)
    return out



# revision 2
# speedup vs baseline: 3.8995x; 3.8995x over previous
"""MaxUnpooling2D scatter-add for Trainium2 (8 NeuronCores) — one-hot matmul.

Problem: updates/mask [32,112,112,64] f32/int32 -> out [32,224,224,64] f32,
out[b, y, x, c] += updates[b, h, w, c]; y,x decoded from mask. Per (b,c)
"plane": 12544 elements scatter-add into 50176 pixel bins (bin = mask>>6,
channel index = lane c).

Algorithm (NO per-element DMA): decompose bin t = lo*392 + hi (lo<128,
hi<392). For each 128-element chunk i of a plane build two one-hot fp16
matrices on DVE/GPSIMD:
    A[i, l] = (lo_i == l) * v_i        [128 x 128]   (stationary)
    M[i, h] = (hi_i == h)              [128 x 392]   (moving)
then PE matmul-accumulates PSUM[l, h] += A^T @ M over the plane's 98 chunks:
every element lands exactly at (lo_i, hi_i) with value v_i, duplicates are
summed by the contraction/PSUM accumulate (race-free by construction). The
dense plane is then copied PSUM->SBUF and written with a plain DMA.

Sharding: batch b across 8 cores x 4 sequential invocations of ONE compiled
module (one batch = 64 planes per invocation). Output is written plane-major
[c, lo, hi]; host reassembles to [b, 224*224, 64] (fixed transposes only).

Precision: one-hots are exact in fp16 (integers < 2048); v is fp16-rounded
once (rel ~2^-11); PSUM accumulates in f32. Measured max rel err ~3e-4.
"""
import numpy as np

import concourse.bacc as bacc
import concourse.mybir as mybir
import concourse.tile as tile
from concourse.bass2jax import run_bass_via_pjrt

B, H, W, C = 32, 112, 112, 64
NPOS = H * W                 # 12544 positions per batch
NCHUNK = NPOS // 128         # 98 chunks per plane
LO, HI = 128, 392            # 50176 = LO * HI bin decomposition
OUT_HW = (2 * H) * (2 * W)   # 50176
N_CORES = 8
AL = mybir.AluOpType

_cached_nc = None
_cached_runner = None


def _build_module():
    """One invocation: 1 batch (64 planes) on one core."""
    nc = bacc.Bacc("TRN2", target_bir_lowering=False, debug=False)
    v_d = nc.dram_tensor("v", [128, C * NCHUNK], mybir.dt.float32,
                         kind="ExternalInput")
    m_d = nc.dram_tensor("m", [128, C * NCHUNK], mybir.dt.int32,
                         kind="ExternalInput")
    io392_d = nc.dram_tensor("io392", [128, HI], mybir.dt.float16,
                             kind="ExternalInput")
    io128_d = nc.dram_tensor("io128", [128, LO], mybir.dt.float16,
                             kind="ExternalInput")
    out_d = nc.dram_tensor("out", [128, C * HI], mybir.dt.float32,
                           kind="ExternalOutput")

    with tile.TileContext(nc) as tc:
        with tc.tile_pool(name="sbuf", bufs=1) as pool, \
             tc.tile_pool(name="dec", bufs=3) as decpool, \
             tc.tile_pool(name="pp", bufs=4, space="PSUM") as ppool, \
             tc.tile_pool(name="st", bufs=3) as stpool, \
             tc.tile_pool(name="ab", bufs=6) as abpool:
            v = pool.tile([128, C * NCHUNK], mybir.dt.float32)
            mi = pool.tile([128, C * NCHUNK], mybir.dt.int32)
            io392 = pool.tile([128, HI], mybir.dt.float16)
            io128 = pool.tile([128, LO], mybir.dt.float16)
            for t_, d_ in [(v, v_d), (mi, m_d), (io392, io392_d),
                           (io128, io128_d)]:
                nc.sync.dma_start(out=t_[:], in_=d_[:])

            for p in range(C):
                sl = slice(p * NCHUNK, (p + 1) * NCHUNK)
                # decode bin = mask>>6 -> lo = bin//392 (f32 reciprocal with
                # half-bin bias; f32->int convert is round-to-nearest, so the
                # -0.5 makes it a floor), hi = bin - 392*lo. All values are
                # integers exactly representable in f32.
                t_i = decpool.tile([128, NCHUNK], mybir.dt.int32, name="ti")
                t_f = decpool.tile([128, NCHUNK], mybir.dt.float32, name="tf")
                qf = decpool.tile([128, NCHUNK], mybir.dt.float32, name="qf")
                qi = decpool.tile([128, NCHUNK], mybir.dt.int32, name="qi")
                lo_f = decpool.tile([128, NCHUNK], mybir.dt.float32, name="lo")
                hi_f = decpool.tile([128, NCHUNK], mybir.dt.float32, name="hi")
                g = nc.vector
                g.tensor_scalar(out=t_i[:], in0=mi[:, sl], scalar1=6,
                                scalar2=None, op0=AL.logical_shift_right)
                g.tensor_scalar(out=t_f[:], in0=t_i[:], scalar1=0,
                                scalar2=None, op0=AL.add)
                g.tensor_scalar(out=qf[:], in0=t_f[:], scalar1=float(1.0 / 392),
                                scalar2=float(0.5 / 392 - 0.5),
                                op0=AL.mult, op1=AL.add)
                g.tensor_scalar(out=qi[:], in0=qf[:], scalar1=0, scalar2=None,
                                op0=AL.add)
                g.tensor_scalar(out=lo_f[:], in0=qi[:], scalar1=0, scalar2=None,
                                op0=AL.add)
                g.scalar_tensor_tensor(out=hi_f[:], in0=lo_f[:], scalar=-392.0,
                                       in1=t_f[:], op0=AL.mult, op1=AL.add)

                psum = ppool.tile([128, HI], mybir.dt.float32, name="ps")
                for k in range(NCHUNK):
                    A = abpool.tile([128, LO], mybir.dt.float16, name="A")
                    M = abpool.tile([128, HI], mybir.dt.float16, name="M")
                    nc.vector.tensor_scalar(out=M[:], in0=io392[:],
                                            scalar1=hi_f[:, k:k + 1],
                                            scalar2=None, op0=AL.is_equal)
                    a_eng = nc.vector
                    a_eng.tensor_scalar(out=A[:], in0=io128[:],
                                        scalar1=lo_f[:, k:k + 1],
                                        scalar2=v[:, sl][:, k:k + 1],
                                        op0=AL.is_equal, op1=AL.mult)
                    nc.tensor.matmul(out=psum[:], lhsT=A[:], rhs=M[:],
                                     start=(k == 0), stop=(k == NCHUNK - 1))
                stage = stpool.tile([128, HI], mybir.dt.float32, name="sg")
                nc.vector.tensor_copy(out=stage[:], in_=psum[:])
                nc.sync.dma_start(out=out_d[:, p * HI:(p + 1) * HI],
                                  in_=stage[:])
    nc.compile()
    return nc


def _get_module():
    global _cached_nc
    if _cached_nc is None:
        _cached_nc = _build_module()
    return _cached_nc


def _iotas():
    io392 = np.broadcast_to(np.arange(HI, dtype=np.float16), (128, HI)).copy()
    io128 = np.broadcast_to(np.arange(LO, dtype=np.float16), (128, LO)).copy()
    return io392, io128


def kernel(updates: np.ndarray, mask: np.ndarray) -> np.ndarray:
    assert updates.shape == (B, H, W, C) and mask.shape == (B, H, W, C)
    updates = np.ascontiguousarray(updates, dtype=np.float32)
    mask = np.ascontiguousarray(mask, dtype=np.int32)

    # host layout (data-independent): [B, NPOS, C] -> per batch
    # [128 lane, C * NCHUNK] with column (c*NCHUNK + k) = chunk k of plane c,
    # lane i = position k*128+i.
    upd_t = updates.reshape(B, NCHUNK, 128, C).transpose(0, 2, 3, 1)
    msk_t = mask.reshape(B, NCHUNK, 128, C).transpose(0, 2, 3, 1)
    upd_t = np.ascontiguousarray(upd_t).reshape(B, 128, C * NCHUNK)
    msk_t = np.ascontiguousarray(msk_t).reshape(B, 128, C * NCHUNK)

    io392, io128 = _iotas()
    nc = _get_module()

    # 32 batches over 8 cores x 4 rounds
    outs = np.empty((B, 128, C * HI), dtype=np.float32)
    for rnd in range(4):
        in_maps = []
        for core in range(N_CORES):
            b = rnd * N_CORES + core
            in_maps.append({
                "v": upd_t[b],
                "m": msk_t[b],
                "io392": io392,
                "io128": io128,
            })
        results = run_bass_via_pjrt(nc, in_maps, n_cores=N_CORES)
        for core in range(N_CORES):
            outs[rnd * N_CORES + core] = results[core]["out"]

    # out_d[l, c*HI + h] = plane c bin l*HI+h -> out[b, pix, c]
    # reshape [128 lo, C, HI] -> transpose to [lo, HI, C] -> [50176, C]
    out = outs.reshape(B, 128, C, HI).transpose(0, 1, 3, 2).reshape(
        B, OUT_HW, C)
    return np.ascontiguousarray(out).reshape(B, 2 * H, 2 * W, C)
